# revision 9
# baseline (speedup 1.0000x reference)
"""Trainium2 Bass kernel for nn_CrossAttention (B=2, S=4096, D=512, H=8, dh=64).

Sharding over 8 cores: core c handles batch b=c//4 and head pair p=c%4
(heads 2p, 2p+1).  Each core:
  - computes Q^T, K^T (its 2 heads) and V from x_b via PE matmuls
    (x fed pre-transposed and pre-cast to bf16 from host, so the contraction
    dim is on partitions)
  - flash-style attention: scores computed transposed ([k, q] layout) so the
    exp'd scores feed att@V directly as the moving operand; row-sums obtained
    by augmenting V with a ones column; normalization via a PE broadcast of
    1/rowsum.  The two heads run as independent interleaved streams so the
    tensor engine always has dense work (keeps the PE HAM clock-gate warm).
  - partial output projection out^T @ Wo_slice^T -> [S, 512] partial (f32r).
Host sums the 4 partials per batch and adds the bias.
"""
import sys

sys.path.insert(0, "/opt/trn_rl_repo")

import numpy as np
import ml_dtypes

import concourse.bass as bass  # noqa: F401  (import side effects)
from concourse import bacc, mybir
import concourse.tile as tile
from concourse.bass_utils import run_bass_kernel_spmd
from concourse.masks import make_identity

B, S, D = 2, 4096, 512
H, DH = 8, 64
N_CORES = 8
F32 = mybir.dt.float32
F32R = mybir.dt.float32r
BF16 = mybir.dt.bfloat16

CH = 1024          # q chunk: scores/exp granularity and per-head accumulator width
SCALE = 1.0 / 8.0  # 1/sqrt(dh)

_cache = {}


def _build():
    if "nc" in _cache:
        return _cache["nc"]

    nc = bacc.Bacc(None, target_bir_lowering=False, debug=False)
    xT_d = nc.dram_tensor("xT", [D, S], BF16, kind="ExternalInput")
    wqkvT_d = nc.dram_tensor("wqkvT", [D, 384], BF16, kind="ExternalInput")
    woT_d = nc.dram_tensor("woT", [128, D], F32, kind="ExternalInput")
    out_d = nc.dram_tensor("out", [S, D], F32, kind="ExternalOutput")

    with tile.TileContext(nc) as tc:
        with (
            tc.tile_pool(name="const", bufs=1) as const,
            tc.tile_pool(name="persist", bufs=1) as persist,
            tc.tile_pool(name="psum", bufs=1, space="PSUM") as psum,
        ):
            ident = const.tile([128, 128], BF16, tag="ident")
            make_identity(nc, ident)
            ones_f = const.tile([128, 64], F32, tag="ones_f")
            nc.vector.memset(ones_f, 1.0)
            ones = const.tile([128, 64], F32R, tag="ones")
            nc.vector.tensor_copy(ones, ones_f)
            woT = const.tile([128, D], F32R, tag="woT")
            nc.sync.dma_start(out=woT, in_=woT_d[:].bitcast(F32R))

            # warm up the PE HAM clock gate with a dense burst of dummy matmuls
            wsrc = const.tile([128, 512], BF16, tag="wsrc")
            nc.vector.memset(wsrc, 0.125)
            for _ in range(12):
                wps = psum.tile([128, 512], F32, tag="sc", bufs=2)
                nc.tensor.matmul(wps, lhsT=wsrc[:, 0:128], rhs=wsrc, start=True, stop=True)

            # persistent activations (live through the whole kernel)
            qT = persist.tile([128, S], BF16, tag="qT")  # rows 0-63 head0, 64-127 head1
            kT = persist.tile([128, S], BF16, tag="kT")
            vaug = persist.tile([128, 32 * 130], BF16, tag="vaug")
            outT = persist.tile([128, S], F32R, tag="outT")

            # ---------------- phase 1: QKV projections + V transpose ----------
            with tc.tile_pool(name="phase1", bufs=1) as p1:
                xT = []
                for e in range(4):
                    t = p1.tile([128, S], BF16, tag=f"xT{e}")
                    nc.sync.dma_start(out=t, in_=xT_d[e * 128:(e + 1) * 128, :])
                    xT.append(t)
                wqkvT = []
                for e in range(4):
                    t = p1.tile([128, 384], BF16, tag=f"wqkvT{e}")
                    nc.sync.dma_start(out=t, in_=wqkvT_d[e * 128:(e + 1) * 128, :])
                    wqkvT.append(t)
                vT = p1.tile([128, S], BF16, tag="vT")

                # per-(ktile, head) ones columns of vaug (offsets 64 and 129 in
                # each 130-wide block): strided copy from the fp32 ones tile
                vaug_k = vaug.rearrange("p (k c) -> p k c", c=130)
                nc.vector.tensor_copy(vaug_k[:, :, 64], ones_f[:, 0:32])
                nc.vector.tensor_copy(vaug_k[:, :, 129], ones_f[:, 0:32])

                for g, dest in enumerate((qT, kT, vT)):
                    for sg in range(4):  # s groups of 1024
                        ps = psum.tile([128, CH], F32, tag=f"acc{sg % 2}", name=f"qkv{g}_{sg}")
                        for e in range(4):
                            for j in range(2):
                                nc.tensor.matmul(
                                    ps[:, j * 512:(j + 1) * 512],
                                    lhsT=wqkvT[e][:, g * 128:(g + 1) * 128],
                                    rhs=xT[e][:, (sg * 2 + j) * 512:(sg * 2 + j + 1) * 512],
                                    start=(e == 0),
                                    stop=(e == 3),
                                )
                        nc.vector.tensor_copy(dest[:, sg * CH:(sg + 1) * CH], ps)

                # transpose V^T [128, S] -> natural V layout inside vaug
                for kt in range(32):
                    tp = psum.tile([128, 128], BF16, tag="sc", bufs=2)
                    nc.tensor.transpose(tp, vT[:, kt * 128:(kt + 1) * 128], ident)
                    nc.vector.tensor_copy(vaug[:, kt * 130:kt * 130 + 64], tp[:, 0:64])
                    nc.vector.tensor_copy(vaug[:, kt * 130 + 65:kt * 130 + 129], tp[:, 64:128])

            # ---------------- phase 2: attention (interleaved head streams) ---
            with (
                tc.tile_pool(name="work", bufs=3) as work,
                tc.tile_pool(name="bcp", bufs=2) as bcp,
            ):
                for qb in range(S // CH):
                    q0 = qb * CH
                    accs = [
                        psum.tile([65, CH], F32, tag=f"acc{h}", name=f"acc{h}_{qb}")
                        for h in range(2)
                    ]
                    for kt in range(32):
                        for h in range(2):
                            bp = h * 64
                            acc = accs[h]
                            vs = vaug[:, kt * 130 + h * 65:kt * 130 + h * 65 + 65]
                            sc = psum.tile([128, CH], F32, tag="sc", bufs=2)
                            for j in range(2):
                                nc.tensor.matmul(
                                    sc[:, j * 512:(j + 1) * 512],
                                    lhsT=kT[bp:bp + 64, kt * 128:(kt + 1) * 128],
                                    rhs=qT[bp:bp + 64, q0 + j * 512:q0 + (j + 1) * 512],
                                    start=True,
                                    stop=True,
                                )
                            ex = work.tile([128, CH], BF16, tag="expT")
                            nc.scalar.activation(
                                ex, sc, mybir.ActivationFunctionType.Exp, scale=SCALE
                            )
                            for j in range(2):
                                nc.tensor.matmul(
                                    acc[:, j * 512:(j + 1) * 512],
                                    lhsT=vs,
                                    rhs=ex[:, j * 512:(j + 1) * 512],
                                    start=(kt == 0),
                                    stop=(kt == 31),
                                )
                    # normalize: outT[h*64:(h+1)*64, q0:q0+CH] = acc[0:64] / acc[64]
                    for h in range(2):
                        acc = accs[h]
                        sums = work.tile([128, CH], F32, tag="sums")
                        nc.vector.tensor_copy(sums[64:65, :], acc[64:65, :])
                        sums0 = work.tile([1, CH], F32, tag="sums0")
                        nc.sync.dma_start(out=sums0, in_=sums[64:65, :])
                        recip_f = work.tile([1, CH], F32, tag="recip_f")
                        nc.vector.reciprocal_approx_fast(recip_f, sums0)
                        recip = work.tile([1, CH], F32R, tag="recip")
                        nc.vector.tensor_copy(recip, recip_f)
                        bc = psum.tile([64, CH], F32, tag="sc", bufs=2)
                        for j in range(2):
                            nc.tensor.matmul(
                                bc[:, j * 512:(j + 1) * 512],
                                lhsT=ones[0:1, :],
                                rhs=recip[0:1, j * 512:(j + 1) * 512],
                                start=True,
                                stop=True,
                            )
                        bcs = bcp.tile([64, CH], F32, tag="bcs")
                        nc.vector.tensor_copy(bcs, bc)
                        if h == 0:
                            nc.vector.tensor_mul(outT[0:64, q0:q0 + CH], acc[0:64, :], bcs)
                        else:
                            tmp = bcp.tile([64, CH], F32R, tag="tmp")
                            nc.vector.tensor_mul(tmp, acc[0:64, :], bcs)
                            nc.sync.dma_start(out=outT[64:128, q0:q0 + CH], in_=tmp)

                # ---------------- phase 3: output projection ------------------
                for st in range(32):
                    pj = psum.tile([128, D], F32, tag="sc", bufs=2)
                    nc.tensor.matmul(
                        pj,
                        lhsT=outT[:, st * 128:(st + 1) * 128],
                        rhs=woT,
                        start=True,
                        stop=True,
                    )
                    po = work.tile([128, D], F32, tag="po")
                    nc.vector.tensor_copy(po, pj)
                    nc.sync.dma_start(out=out_d[st * 128:(st + 1) * 128, :], in_=po)

    nc.compile()
    _cache["nc"] = nc
    return nc


def _prep_inputs(hidden_states, Wq, Wk, Wv, Wo):
    bf = ml_dtypes.bfloat16
    xT = [np.ascontiguousarray(hidden_states[b].T).astype(bf) for b in range(B)]
    in_maps = []
    for c in range(N_CORES):
        b = c // 4
        p = c % 4
        rows = slice(p * 128, (p + 1) * 128)
        wpack = np.concatenate([Wq[rows], Wk[rows], Wv[rows]], axis=0)  # [384, 512]
        in_maps.append(
            {
                "xT": xT[b],
                "wqkvT": np.ascontiguousarray(wpack.T).astype(bf),
                "woT": np.ascontiguousarray(Wo[:, rows].T),
            }
        )
    return in_maps


def kernel(hidden_states, Wq, Wk, Wv, Wo, bo):
    hidden_states = np.asarray(hidden_states, dtype=np.float32)
    Wq = np.asarray(Wq, dtype=np.float32)
    Wk = np.asarray(Wk, dtype=np.float32)
    Wv = np.asarray(Wv, dtype=np.float32)
    Wo = np.asarray(Wo, dtype=np.float32)
    bo = np.asarray(bo, dtype=np.float32)

    nc = _build()
    in_maps = _prep_inputs(hidden_states, Wq, Wk, Wv, Wo)
    res = run_bass_kernel_spmd(nc, in_maps, list(range(N_CORES)))
    out = np.zeros((B, S, D), dtype=np.float32)
    for c in range(N_CORES):
        out[c // 4] += res.results[c]["out"]
    out += bo[None, None, :]
    return out
